# revision 5
# baseline (speedup 1.0000x reference)
"""Trainium2 Bass kernel for windowed/global sparse attention (Swin-style
relative-position bias + 1 global token), data-parallel over batch on 8 cores.

Shapes: B=16, N=785 (1 global + 28x28 local), C=768, H=12 heads, d=64.

Per-core device program (2 batches/core, software-pipelined):
  - qT/kT computed transposed ([d, tokens]) so S^T = K @ Q^T needs no
    transposes anywhere; v computed natural ([tokens, d]) with a ones column
    appended per head so the P @ V matmul also yields softmax denominators.
  - softmax: exp(S + bias) = exp(S) * expB with expB = exp(bias) gathered on
    host at constant indices and shipped bf16; the two heads of a pair write
    one fused [128, 2W] SBUF exp tile so the expB multiply is a single
    2x-rate DVE op.
  - PSUM: 3 rotating 2-bank slots dedicated to S tiles + 2 rotating 1-bank
    slots for everything else (O/v/proj/qkv convoys split per column-group so
    each fits one bank) — the S/exp pipeline never stalls behind filler
    convoys, and all evacuations pipeline across the two small slots.
  - normalization: denominators from all 12 heads staged to DRAM, one batched
    DVE reciprocal, DMA-broadcast back to [128, N], multiplied into O^T;
    proj consumes O^T directly as lhsT.
  - schedule: [qkv0 || v0] dense, attention-0 with x1/qkv1 fillers,
    v1 at the seam, attention-1 with proj0 fillers, then norm1+proj1 —
    keeps the PE activity monitor from re-throttling the clock during the
    exp-paced attention stretches.
"""

import numpy as np
import ml_dtypes

import concourse.bass as bass
import concourse.bacc as bacc
import concourse.tile as tile
from concourse.tile import add_dep_helper
from concourse import mybir
from concourse.bass_utils import run_bass_kernel_spmd

F32 = mybir.dt.float32
BF16 = mybir.dt.bfloat16

WX = WY = 28
NGLO = 1
H = 12
L = WX * WY            # 784
N = NGLO + L           # 785
C = 768
HD = C // H            # 64
SCALE = HD ** -0.5
B = 16
N_CORES = 8
B_LOC = B // N_CORES   # 2
NCC = C // 128         # 6 contraction chunks
NKC = (N + 127) // 128  # 7 key/token chunks (last = 17 rows)
NPAIR = H // 2         # 6 head pairs
W = 786                # padded free width for N-sized tiles (even, 4B-aligned)
W2 = 2 * W

CG_N = [(0, 512), (512, 274)]
CG_C = [(0, 512), (512, 256)]


def _kr(kc):
    return min(128, N - kc * 128)


def build_nc():
    nc = bacc.Bacc(None, target_bir_lowering=False)

    xT_d = nc.dram_tensor("xT", [B_LOC, C, N], BF16, kind="ExternalInput")
    qkvwT_d = nc.dram_tensor("qkv_wT", [C, 3 * C], BF16, kind="ExternalInput")
    pwT_d = nc.dram_tensor("proj_wT", [C, C], BF16, kind="ExternalInput")
    pb_d = nc.dram_tensor("proj_b", [1, C], F32, kind="ExternalInput")
    expB_d = nc.dram_tensor("expB", [H, N, N], BF16, kind="ExternalInput")
    out_d = nc.dram_tensor("out", [B_LOC, N, C], F32, kind="ExternalOutput")
    dinv_d = nc.dram_tensor("dinv_scratch", [B_LOC, H, N], F32)

    with tile.TileContext(nc) as tc:
        with (
            tc.tile_pool(name="consts", bufs=1) as consts,
            tc.tile_pool(name="perb", bufs=2) as perb,
            tc.tile_pool(name="expbp", bufs=3) as expbp,
            tc.tile_pool(name="flow", bufs=3) as flow,
            tc.tile_pool(name="ptp", bufs=8) as ptp,
            tc.tile_pool(name="norm", bufs=1) as norm,
            tc.tile_pool(name="outp", bufs=2) as outp,
            tc.tile_pool(name="psum_s", bufs=3, space=bass.MemorySpace.PSUM) as psum_s,
            tc.tile_pool(name="psum_o", bufs=2, space=bass.MemorySpace.PSUM) as psum_o,
        ):
            # ---- weights (resident, bf16); proj weights loaded last ----
            qkvw = []
            for cc in range(NCC):
                t = consts.tile([128, 3 * C], BF16, tag=f"qkvw{cc}", name=f"qkvw{cc}")
                qkvw.append(t)
            pw16 = []
            for cc in range(NCC):
                t = consts.tile([128, C], BF16, tag=f"pw{cc}", name=f"pw{cc}")
                pw16.append(t)
            pb_rep = consts.tile([128, C], F32, tag="pbrep")

            def emit_weight_loads_qkv():
                for cc in range(NCC):
                    nc.sync.dma_start(
                        qkvw[cc][:], qkvwT_d[cc * 128:(cc + 1) * 128, :]
                    )

            def emit_weight_loads_proj():
                for cc in range(NCC):
                    nc.sync.dma_start(
                        pw16[cc][:], pwT_d[cc * 128:(cc + 1) * 128, :]
                    )
                nc.sync.dma_start(pb_rep[:], pb_d[:].to_broadcast([128, C]))

            def emit_x(b):
                # pad column [N:W] left as garbage: it only ever feeds the
                # q=785 / token=785 output columns, which are never read.
                xts = []
                for cc in range(NCC):
                    t = perb.tile([128, W], BF16, tag=f"xt{cc}", name=f"xt{cc}_{b}")
                    nc.sync.dma_start(
                        t[:, 0:N], xT_d[b, cc * 128:(cc + 1) * 128, :]
                    )
                    xts.append(t)
                return xts

            def emit_qkvT_cg(b, xts, oc, qT, kT, c0, cn, evac_vector):
                """one column-group convoy of the q/k projection for output
                chunk oc, in a 1-bank psum slot."""
                ps = psum_o.tile([128, 512], F32, tag="o",
                                 name=f"psqk{oc}_{c0}_{b}")
                first = None
                for cc in range(NCC):
                    mm = nc.tensor.matmul(
                        ps[:, 0:cn],
                        qkvw[cc][:, oc * 128:(oc + 1) * 128],
                        xts[cc][:, c0:c0 + cn],
                        start=(cc == 0),
                        stop=(cc == NCC - 1),
                    )
                    if first is None:
                        first = mm
                dst = qT[oc] if oc < NCC else kT[oc - NCC]
                if evac_vector:
                    nc.vector.tensor_copy(dst[:, c0:c0 + cn], ps[:, 0:cn])
                else:
                    nc.scalar.copy(dst[:, c0:c0 + cn], ps[:, 0:cn])
                return first

            def emit_qkvT_chunk(b, xts, j, qT, kT, evac_vector):
                """produce qT[j] and kT[j] for batch b (4 cg convoys)."""
                firsts = []
                for oc in (j, NCC + j):
                    for (c0, cn) in CG_N:
                        f = emit_qkvT_cg(b, xts, oc, qT, kT, c0, cn,
                                         evac_vector)
                        firsts.append(f)
                return firsts

            def emit_v_cg(b, xts, kc, vp, ci, evac_vector):
                """one column-group convoy of the V projection for key chunk
                kc; cg boundary at 512 = head 8 boundary."""
                kr = _kr(kc)
                c0, cn = CG_C[ci]
                h0 = c0 // (HD)        # first head in this cg (8 or 0)
                nh = cn // HD          # heads in this cg
                ps = psum_o.tile([128, 512], F32, tag="o",
                                 name=f"psv{kc}_{ci}_{b}")
                first = None
                for cc in range(NCC):
                    mm = nc.tensor.matmul(
                        ps[0:kr, 0:cn],
                        xts[cc][:, kc * 128:kc * 128 + kr],
                        qkvw[cc][:, 2 * C + c0:2 * C + c0 + cn],
                        start=(cc == 0),
                        stop=(cc == NCC - 1),
                    )
                    if first is None:
                        first = mm
                v3 = vp[kc][:].rearrange("p (h e) -> p h e", e=HD + 1)
                src3 = ps[0:kr, 0:cn].rearrange("p (h d) -> p h d", d=HD)
                if evac_vector:
                    nc.vector.tensor_copy(v3[0:kr, h0:h0 + nh, 0:HD], src3)
                else:
                    nc.scalar.copy(v3[0:kr, h0:h0 + nh, 0:HD], src3)
                if ci == 1:
                    nc.vector.memset(v3[0:kr, :, HD:HD + 1], 1.0)
                return first

            def alloc_vp(b):
                return [perb.tile([128, H * (HD + 1)], BF16, tag=f"vp{i}",
                                  name=f"vp{i}_{b}") for i in range(NKC)]

            def alloc_oT(b):
                return [perb.tile([128, W], BF16, tag=f"oT{i}", name=f"oT{i}_{b}")
                        for i in range(NCC)]

            def emit_attn_pass1(b, j, qT, kT):
                """S + exp + fused expB-multiply for head pair (2j, 2j+1).
                The two heads' S matmuls sit on disjoint PE row groups; their
                exps write one fused [128, 2W] tile so the expB multiply is a
                single 2x DVE op."""
                pts = [None] * NKC
                pacer = None
                for kc in range(NKC):
                    kr = _kr(kc)
                    ps_pair = [
                        psum_s.tile([128, W], F32, tag="s",
                                    name=f"pss{2 * j + hh}_{kc}_{b}")
                        for hh in range(2)
                    ]
                    for (c0, cn) in CG_N:
                        for hh in range(2):
                            po = hh * 64
                            mm = nc.tensor.matmul(
                                ps_pair[hh][0:kr, c0:c0 + cn],
                                kT[j][po:po + 64, kc * 128:kc * 128 + kr],
                                qT[j][po:po + 64, c0:c0 + cn],
                                start=True,
                                stop=True,
                            )
                            if kc == 2 and pacer is None:
                                pacer = mm
                    ebt = expbp.tile([128, W2], BF16, tag="expb",
                                     name=f"ebt{j}_{kc}_{b}")
                    for hh in range(2):
                        h = 2 * j + hh
                        nc.sync.dma_start(
                            ebt[0:kr, hh * W:hh * W + N],
                            expB_d[h, kc * 128:kc * 128 + kr, :],
                        )
                    es = flow.tile([128, W2], BF16, tag="expS",
                                   name=f"es{j}_{kc}_{b}")
                    for hh in range(2):
                        nc.scalar.activation(
                            es[0:kr, hh * W:(hh + 1) * W],
                            ps_pair[hh][0:kr, 0:W],
                            mybir.ActivationFunctionType.Exp,
                        )
                    pt = ptp.tile([128, W2], BF16, tag="pT",
                                  name=f"pt{j}_{kc}_{b}")
                    nc.vector.tensor_tensor(
                        pt[0:kr, 0:W2],
                        es[0:kr, 0:W2],
                        ebt[0:kr, 0:W2],
                        mybir.AluOpType.mult,
                    )
                    pts[kc] = pt
                return pts, pacer

            def emit_attn_pass2(b, j, pts, vp, oT, dall):
                """O-accumulation convoys for head pair (2j, 2j+1); each
                column group runs in its own 1-bank slot so the cg0 evac
                overlaps the cg1 convoy."""
                for hh in range(2):
                    h = 2 * j + hh
                    dn = norm.tile([65, W], F32, tag="dn", bufs=1,
                                   name=f"dn{h}_{b}")
                    for (c0, cn) in CG_N:
                        ps_o = psum_o.tile([128, 512], F32, tag="o",
                                           name=f"pso{h}_{c0}_{b}")
                        for kc in range(NKC):
                            kr = _kr(kc)
                            nc.tensor.matmul(
                                ps_o[0:HD + 1, 0:cn],
                                vp[kc][0:kr, h * (HD + 1):(h + 1) * (HD + 1)],
                                pts[kc][0:kr, hh * W + c0:hh * W + c0 + cn],
                                start=(kc == 0),
                                stop=(kc == NKC - 1),
                            )
                        nn = min(cn, N - c0)
                        nc.vector.tensor_copy(
                            oT[j][hh * 64:hh * 64 + 64, c0:c0 + nn],
                            ps_o[0:64, 0:nn],
                        )
                        nc.vector.tensor_copy(
                            dn[64:65, c0:c0 + nn], ps_o[64:65, 0:nn]
                        )
                    nc.sync.dma_start(dall[h:h + 1, 0:N], dn[64:65, 0:N])

            def emit_norm(b, oT, dall):
                # batched reciprocal + DMA broadcast + in-place normalize
                dinv = norm.tile([12, W], F32, tag="dinv", name=f"dinv_{b}")
                nc.vector.reciprocal(dinv[0:H, 0:N], dall[0:H, 0:N])
                nc.sync.dma_start(dinv_d[b], dinv[0:H, 0:N])
                for cc in range(NCC):
                    dr = norm.tile([128, W], F32, tag="drep", bufs=2,
                                   name=f"dr{cc}_{b}")
                    for hh in range(2):
                        row = dinv_d[b, 2 * cc + hh, :]
                        src = bass.AP(
                            tensor=row.tensor, offset=row.offset,
                            ap=[[0, 64]] + row.ap,
                        )
                        nc.sync.dma_start(dr[hh * 64:(hh + 1) * 64, 0:N], src)
                    nc.vector.tensor_tensor(
                        oT[cc][:, 0:N], oT[cc][:, 0:N], dr[:, 0:N],
                        mybir.AluOpType.mult,
                    )
                return oT

            def emit_proj_chunk(b, oT, tt):
                ts_ = _kr(tt)
                pfirsts = []
                ob = outp.tile([128, C], F32, tag="ob", name=f"ob{tt}_{b}")
                for (c0, cn) in CG_C:
                    ps = psum_o.tile([128, 512], F32, tag="o",
                                     name=f"psp{tt}_{c0}_{b}")
                    for cc in range(NCC):
                        mm = nc.tensor.matmul(
                            ps[0:ts_, 0:cn],
                            oT[cc][:, tt * 128:tt * 128 + ts_],
                            pw16[cc][:, c0:c0 + cn],
                            start=(cc == 0),
                            stop=(cc == NCC - 1),
                        )
                        if cc == 0:
                            pfirsts.append(mm)
                    nc.vector.tensor_tensor(
                        ob[0:ts_, c0:c0 + cn], ps[0:ts_, 0:cn],
                        pb_rep[0:ts_, c0:c0 + cn],
                        mybir.AluOpType.add,
                    )
                nc.sync.dma_start(
                    out_d[b, tt * 128:tt * 128 + ts_, :], ob[0:ts_, :]
                )
                return pfirsts

            # ---- phase A: batch-0 qkv + v, interleaved, PE-dense ----
            xts0 = emit_x(0)
            emit_weight_loads_qkv()
            qT0 = [perb.tile([128, W], BF16, tag=f"qT{i}", name=f"qT{i}_0")
                   for i in range(NCC)]
            kT0 = [perb.tile([128, W], BF16, tag=f"kT{i}", name=f"kT{i}_0")
                   for i in range(NCC)]
            vp0 = alloc_vp(0)
            for i in range(NKC):
                if i < NCC:
                    emit_qkvT_chunk(0, xts0, i, qT0, kT0, evac_vector=False)
                for ci in range(2):
                    emit_v_cg(0, xts0, i, vp0, ci, evac_vector=False)
            emit_weight_loads_proj()

            # ---- phase B: batch-0 attention; x1/qkv1 as PE filler ----
            oT0 = alloc_oT(0)
            dall0 = norm.tile([12, W], F32, tag="dall", bufs=2, name="dall_0")
            xts1 = emit_x(1)
            qT1 = [perb.tile([128, W], BF16, tag=f"qT{i}", name=f"qT{i}_1")
                   for i in range(NCC)]
            kT1 = [perb.tile([128, W], BF16, tag=f"kT{i}", name=f"kT{i}_1")
                   for i in range(NCC)]
            vp1 = alloc_vp(1)
            pacers0 = []
            pend0 = []
            for j in range(NPAIR):
                if j >= 1:
                    emit_attn_pass2(0, j - 1, pend0[j - 1], vp0, oT0, dall0)
                    # filler: one qkv1 chunk per pair-phase (1-bank slots)
                    qf = emit_qkvT_chunk(1, xts1, j - 1, qT1, kT1,
                                         evac_vector=True)
                    for f in qf:
                        add_dep_helper(f.ins, pacers0[j - 1].ins, sync=False,
                                       reason="pace qkvT1 filler")
                pts_j, pac = emit_attn_pass1(0, j, qT0, kT0)
                pacers0.append(pac)
                pend0.append(pts_j)
            emit_attn_pass2(0, NPAIR - 1, pend0[NPAIR - 1], vp0, oT0, dall0)
            # seam: last qkv1 chunk + all v1 convoys + norm0
            qf = emit_qkvT_chunk(1, xts1, NCC - 1, qT1, kT1, evac_vector=True)
            for f in qf:
                add_dep_helper(f.ins, pacers0[NPAIR - 1].ins, sync=False,
                               reason="pace qkvT1 tail")
            for kc in range(NKC):
                for ci in range(2):
                    vf = emit_v_cg(1, xts1, kc, vp1, ci, evac_vector=True)
                    if kc < 4:
                        add_dep_helper(vf.ins, pacers0[2 + kc].ins, sync=False,
                                       reason="pace v1 filler")
            emit_norm(0, oT0, dall0)

            # ---- phase C: batch-1 attention; proj0 as PE filler ----
            oT1 = alloc_oT(1)
            dall1 = norm.tile([12, W], F32, tag="dall", bufs=2, name="dall_1")
            pacers1 = []
            pend1 = []
            for j in range(NPAIR):
                if j >= 1:
                    emit_attn_pass2(1, j - 1, pend1[j - 1], vp1, oT1, dall1)
                pts_j, pac = emit_attn_pass1(1, j, qT1, kT1)
                pacers1.append(pac)
                pend1.append(pts_j)
                if j >= 1:
                    pf = emit_proj_chunk(0, oT0, j - 1)
                    for f in pf:
                        add_dep_helper(f.ins, pacers1[j - 1].ins, sync=False,
                                       reason="pace proj0 filler")
            emit_attn_pass2(1, NPAIR - 1, pend1[NPAIR - 1], vp1, oT1, dall1)
            for tt in range(NPAIR - 1, NKC):
                pf = emit_proj_chunk(0, oT0, tt)
                for f in pf:
                    add_dep_helper(f.ins, pacers1[NPAIR - 1].ins, sync=False,
                                   reason="pace proj0 tail")
            emit_norm(1, oT1, dall1)
            for tt in range(NKC):
                emit_proj_chunk(1, oT1, tt)

    nc.compile()
    return nc


def _relative_position_index():
    coords = np.stack(np.meshgrid(np.arange(WX), np.arange(WY), indexing="ij"))
    cf = coords.reshape(2, -1)
    rel = cf[:, :, None] - cf[:, None, :]
    rel = rel.transpose(1, 2, 0).astype(np.int64)
    rel[:, :, 0] += WX - 1
    rel[:, :, 1] += WY - 1
    rel[:, :, 0] *= 2 * WY - 1
    return rel.sum(-1)  # [L, L]


def _host_prep(x, qkv_w, proj_w, proj_b, rel_table, g2l, g2g):
    x = np.asarray(x, np.float32)
    qkv_w = np.asarray(qkv_w, np.float32)
    proj_w = np.asarray(proj_w, np.float32)
    proj_b = np.asarray(proj_b, np.float32)
    rel_table = np.asarray(rel_table, np.float32)
    g2l = np.asarray(g2l, np.float32)
    g2g = np.asarray(g2g, np.float32)

    bf16 = ml_dtypes.bfloat16
    xT = np.ascontiguousarray(x.transpose(0, 2, 1)).astype(bf16)   # [B, C, N]
    qkv_wT = np.ascontiguousarray(qkv_w.T).copy()                  # [C, 3C]
    qkv_wT[:, :C] *= SCALE                                         # fold q scale
    qkv_wT = qkv_wT.astype(bf16)
    proj_wT = np.ascontiguousarray(proj_w.T).astype(bf16)          # [C, C]
    pb = proj_b.reshape(1, C)

    # expB[h, k, q] = exp(bias[h, q, k]); exp applied at table granularity,
    # then expanded by the constant-index relative-position gather.
    ridx = _relative_position_index()
    et = np.exp(rel_table)                                         # [3025, H]
    eg2l = np.exp(g2l)                                             # [2, H, 1]
    eg2g = np.exp(g2g)                                             # [H, 1, 1]
    expB = np.empty((H, N, N), np.float32)
    expB[:, 1:, 1:] = et[ridx].transpose(2, 1, 0)                  # [H, k, q]
    expB[:, 0, 0] = eg2g[:, 0, 0]
    expB[:, 1:, 0] = eg2l[0][:, 0][None, :].T                      # global query
    expB[:, 0, 1:] = eg2l[1][:, 0][:, None]                        # global key
    expB16 = expB.astype(bf16)

    in_maps = []
    for i in range(N_CORES):
        in_maps.append({
            "xT": xT[i * B_LOC:(i + 1) * B_LOC],
            "qkv_wT": qkv_wT,
            "proj_wT": proj_wT,
            "proj_b": pb,
            "expB": expB16,
        })
    return in_maps


_NC = None


def get_nc():
    global _NC
    if _NC is None:
        _NC = build_nc()
    return _NC


def kernel(x, qkv_w, proj_w, proj_b, rel_table, g2l, g2g):
    in_maps = _host_prep(x, qkv_w, proj_w, proj_b, rel_table, g2l, g2g)
    nc = get_nc()
    res = run_bass_kernel_spmd(nc, in_maps, core_ids=list(range(N_CORES)))
    out = np.concatenate([res.results[i]["out"] for i in range(N_CORES)], axis=0)
    return out.astype(np.float32)
